# revision 2
# baseline (speedup 1.0000x reference)
"""Trainium2 Bass kernel for nn_AttentionBlock (B=4, H=W=64, C=256, D=32).

Sharding: 8 shards = 4 samples x 2 query-halves. Each core gets the full
sample (rows reordered so its 2048 query rows come first), computes K and
the fused V@Wo projection for all 4096 keys, and attention + residual for
its 2048 queries. No collectives needed.

v2 over the baseline:
  - PE warm-up burst at t=0 (HAM un-throttles ~3.4us in; running junk
    matmuls during the DMA ramp means real work starts at 2.4 GHz).
  - Scores run 2-way row-tiled (64-deep contraction = 2 replicated D=32
    bands): tiles (0,0)/(64,0) write different PSUM banks concurrently,
    halving score matmul occupancy. Emitted in 2-superstep batches to
    bound tile-mode-switch drains.
  - w_pass deferred (vv=s+6): the qk->scores->exp ramp feeds the scalar
    engine earlier; W projections fill PE slack before attends start.
  - LAG 16 -> 8: the attend stream trails closer, cutting the drain tail.
"""

import sys

if "/opt/trn_rl_repo" not in sys.path:
    sys.path.insert(0, "/opt/trn_rl_repo")

import numpy as np
import ml_dtypes

BF16 = ml_dtypes.bfloat16

# Problem constants
B, HH, WW, C = 4, 64, 64, 256
D = 32
N = HH * WW          # 4096 keys per sample
NQ = N // 2          # 2048 queries per core
NCORES = 8
KC = N // 128        # 32 key chunks

LAG = 8              # attend trails the scores/exp stream by LAG supersteps
WARMUP_MMS = 34      # junk matmuls to trip the HAM clock gate early
TILED_SCORES = True

_compiled_cache = {}


def _build(use_bias: bool):
    from contextlib import ExitStack
    from concourse import bacc, tile, mybir

    f32 = mybir.dt.float32
    bf = mybir.dt.bfloat16
    f8 = mybir.dt.float8e4
    DR = mybir.MatmulPerfMode.DoubleRow

    nc = bacc.Bacc("TRN2", target_bir_lowering=False, debug=False, num_devices=NCORES)

    xT_d = nc.dram_tensor("xT", [128, 2, N], bf, kind="ExternalInput")
    xq32_d = nc.dram_tensor("xq32", [NQ, C], f32, kind="ExternalInput")
    # single blob: [wq0|wq1|wk0|wk1|wvo0|wvo1] so one DMA loads all weights
    wb_d = nc.dram_tensor("wblob", [128, 1024], bf, kind="ExternalInput")
    wbias_d = (
        nc.dram_tensor("wbias", [1, 512], bf, kind="ExternalInput")
        if use_bias
        else None
    )
    out_d = nc.dram_tensor("out", [NQ, C], f32, kind="ExternalOutput")

    Exp = mybir.ActivationFunctionType.Exp
    # undo 2x band replication (64-deep tiled contraction) + 1/sqrt(D)
    SC2 = float(1.0 / (2.0 * np.sqrt(np.float32(D))))
    SC4 = float(1.0 / (4.0 * np.sqrt(np.float32(D))))
    Add = mybir.AluOpType.add
    Mult = mybir.AluOpType.mult

    with tile.TileContext(nc) as tc:
        with ExitStack() as ctx:
            const = ctx.enter_context(tc.tile_pool(name="const", bufs=1))
            big = ctx.enter_context(tc.tile_pool(name="big", bufs=1))
            xbp = ctx.enter_context(tc.tile_pool(name="xbp", bufs=3))
            expp = ctx.enter_context(tc.tile_pool(name="expp", bufs=16))
            small = ctx.enter_context(tc.tile_pool(name="small", bufs=2))
            # PSUM: 3 x [128,1024] working tiles (6 banks; shared by phase-B
            # projections and score matmuls) + 2 pa accumulator banks = 8.
            ps_sc = ctx.enter_context(tc.tile_pool(name="ps_sc", bufs=3, space="PSUM"))
            ps_pa = ctx.enter_context(tc.tile_pool(name="ps_pa", bufs=2, space="PSUM"))

            # ---- PE warm-up: junk matmuls while DMAs stream in ----
            wu = const.tile([128, 128], bf, tag="wu")
            nc.gpsimd.memset(wu[:], 1.0)
            pwarm = ps_sc.tile([128, 1024], f32, tag="sc", name="pwarm")
            for _ in range(WARMUP_MMS):
                nc.tensor.matmul(pwarm[:, 0:128], wu[:], wu[:], start=True, stop=True)

            # ---- weights (one blob DMA) ----
            wall = const.tile([128, 1024], bf, tag="wall")
            nc.sync.dma_start(out=wall[:], in_=wb_d[:])
            wq0 = wall[:, 0:128]
            wq1 = wall[:, 128:256]
            wk0 = wall[:, 256:384]
            wk1 = wall[:, 384:512]
            wvo0 = wall[:, 512:768]
            wvo1 = wall[:, 768:1024]
            if use_bias:
                ones_row = const.tile([1, 512], bf, tag="ones_row")
                nc.gpsimd.memset(ones_row[:], 1.0)
                wbias = const.tile([1, 512], bf, tag="wbias")
                nc.sync.dma_start(out=wbias[:], in_=wbias_d[:])
                wqb = wbias[:, 0:128]
                wkb = wbias[:, 128:256]
                wvob = wbias[:, 256:512]

            # Persistent SBUF: qT replicated across the 4 partition bands of
            # 32 (read as 2 bands of 64 by the tiled score matmuls), kT in 2
            # bands of 64 (band 0: chunks 4s/4s+1, band 1: chunks 4s+2/4s+3),
            # and W (= V@Wo) rows with a ones column at 256 for the softmax
            # denominator.
            qT4 = big.tile([128, NQ], bf, tag="qT4")
            kT4 = big.tile([128, N], bf, tag="kT4")
            wsb = big.tile([128, 16, 2, 272], f8, tag="wsb")
            nc.vector.memset(wsb[:, :, :, 256:257], 1.0)
            expbias = const.tile([128, 1], f32, tag="expbias")
            nc.vector.memset(expbias[:], -2.0)

            # ---- phase B inputs: DMA xT chunks ----
            xbs = []
            for s in range(8):
                xb = xbp.tile([128, 2, 512], bf, tag="xb", bufs=8)
                nc.sync.dma_start(out=xb[:], in_=xT_d[:, :, 512 * s : 512 * s + 512])
                xbs.append(xb)

            def qk_chunk(s):
                # q (chunks 0-3) and k share one [128,1024] psum tile
                p = ps_sc.tile([128, 1024], f32, tag="sc", name=f"pqk{s}")
                if s < 4:
                    nc.tensor.matmul(p[:, 0:512], wq0, xbs[s][:, 0, :], start=True, stop=False)
                    nc.tensor.matmul(p[:, 0:512], wq1, xbs[s][:, 1, :], start=False, stop=not use_bias)
                    if use_bias:
                        nc.tensor.matmul(p[:, 0:512], wqb, ones_row[:], start=False, stop=True)
                nc.tensor.matmul(p[:, 512:1024], wk0, xbs[s][:, 0, :], start=True, stop=False)
                nc.tensor.matmul(p[:, 512:1024], wk1, xbs[s][:, 1, :], start=False, stop=not use_bias)
                if use_bias:
                    nc.tensor.matmul(p[:, 512:1024], wkb, ones_row[:], start=False, stop=True)
                if s < 4:
                    nc.vector.tensor_copy(qT4[:, 512 * s : 512 * s + 512], p[:, 0:512])
                # k carries the softmax scale and the 1/2 (tiled) or 1/4
                # (replicated) band-contraction factor
                if TILED_SCORES:
                    # band 0 <- chunks 4s,4s+1 ; band 1 <- chunks 4s+2,4s+3
                    # (psum k bands are 4 identical replicas, read diagonally)
                    nc.vector.tensor_scalar(
                        kT4[0:64, 256 * s : 256 * s + 256], p[0:64, 512:768], SC2, None, Mult
                    )
                    nc.vector.tensor_scalar(
                        kT4[64:128, 256 * s : 256 * s + 256], p[64:128, 768:1024], SC2, None, Mult
                    )
                else:
                    nc.vector.tensor_scalar(kT4[:, 512 * s : 512 * s + 512], p[:, 512:1024], SC4, None, Mult)

            def w_pass(s):
                # W = x @ (wv@wo): 4 key chunks of 128 per xb in one psum tile
                pw = ps_sc.tile([128, 1024], f32, tag="sc", name=f"pw{s}")
                for j2 in range(4):
                    off = 128 * j2
                    nc.tensor.matmul(pw[:, 256 * j2 : 256 * j2 + 256], xbs[s][:, 0, off : off + 128], wvo0, start=True, stop=False)
                    nc.tensor.matmul(pw[:, 256 * j2 : 256 * j2 + 256], xbs[s][:, 1, off : off + 128], wvo1, start=False, stop=not use_bias)
                    if use_bias:
                        nc.tensor.matmul(pw[:, 256 * j2 : 256 * j2 + 256], ones_row[:, 0:128], wvob, start=False, stop=True)
                # evacuate on vector engine (scalar stays free for exps)
                nc.vector.tensor_copy(wsb[:, 2 * s : 2 * s + 2, :, 0:256], pw[:])

            def scores_mms(g, u):
                pst = ps_sc.tile([128, 1024], f32, tag="sc", name=f"ps{g}_{u}")
                qs = slice(256 * g, 256 * g + 256)
                if TILED_SCORES:
                    # 2-way row tiling: T0 (sbuf 0-63) does chunks 4u,4u+1
                    # into bank A; T8 (sbuf 64-127) does 4u+2,4u+3 into bank
                    # B. Interleave emission for concurrency.
                    k0 = 256 * u
                    nc.tensor.matmul(pst[:, 0:256], kT4[0:64, k0 : k0 + 128], qT4[0:64, qs], start=True, stop=True, tile_position=(0, 0))
                    nc.tensor.matmul(pst[:, 512:768], kT4[64:128, k0 : k0 + 128], qT4[64:128, qs], start=True, stop=True, tile_position=(64, 0))
                    nc.tensor.matmul(pst[:, 256:512], kT4[0:64, k0 + 128 : k0 + 256], qT4[0:64, qs], start=True, stop=True, tile_position=(0, 0))
                    nc.tensor.matmul(pst[:, 768:1024], kT4[64:128, k0 + 128 : k0 + 256], qT4[64:128, qs], start=True, stop=True, tile_position=(64, 0))
                else:
                    for j in range(4):
                        m = 4 * u + j
                        nc.tensor.matmul(
                            pst[:, 256 * j : 256 * j + 256],
                            kT4[:, 128 * m : 128 * m + 128],
                            qT4[:, qs],
                            start=True,
                            stop=True,
                        )
                return pst

            def epilogue(qb, pa_t):
                rec = small.tile([128, 1], f32, tag="rec")
                nc.vector.reciprocal(rec[:], pa_t[:, 256:257])
                xq = small.tile([128, 256], f32, tag="xq", bufs=3)
                nc.sync.dma_start(out=xq[:], in_=xq32_d[128 * qb : 128 * qb + 128, :])
                sc = small.tile([128, 256], f32, tag="sc2")
                nc.vector.tensor_scalar(sc[:], pa_t[:, 0:256], rec[:], None, Mult)
                ot = small.tile([128, 256], f32, tag="ot", bufs=3)
                nc.vector.tensor_tensor(ot[:], sc[:], xq[:], Add)
                nc.sync.dma_start(out=out_d[128 * qb : 128 * qb + 128, :], in_=ot[:])

            # ---- software pipeline ----
            # scores/exp stream order: chunk-diagonal over groups 0-1 during
            # the ramp (both only need q chunk 0 + k chunk u), then
            # group-major. TILED_SCORES emits scores in PAIRS of supersteps
            # (even vv) so the PE switches tile mode at most twice per pair.
            pa_tiles = {}
            ets = {}
            NSS = 8 * 8
            sched_scores = [(g, u) for u in range(8) for g in (0, 1)] + [
                (g, u) for g in range(2, 8) for u in range(8)
            ]
            sched_attend = [(g, u) for g in range(8) for u in range(8)]

            def do_exp(vv):
                g, u = sched_scores[vv]
                pst = pending_scores.pop(vv)
                et = expp.tile([128, 2, 2, 256], f8, tag="e")
                # exp(s - 2): constant shift keeps exp within fp8-e4m3 range
                # (max score ~7 -> e^5 = 148 < 240); the ones-column
                # denominator sees the same shift, so the ratio is exact.
                nc.scalar.activation(et[:], pst[:], Exp, bias=expbias[:])
                ets[(g, u)] = et

            pending_scores = {}
            for vv in range(NSS + LAG):
                if vv < 8:
                    qk_chunk(vv)
                if 6 <= vv < 14:
                    w_pass(vv - 6)
                if vv < NSS:
                    if TILED_SCORES:
                        # pair-batch the (64-row-mode) score matmuls
                        if vv % 2 == 0:
                            pending_scores[vv] = scores_mms(*sched_scores[vv])
                            if vv + 1 < NSS:
                                pending_scores[vv + 1] = scores_mms(*sched_scores[vv + 1])
                    else:
                        pending_scores[vv] = scores_mms(*sched_scores[vv])
                    do_exp(vv)
                va = vv - LAG
                if 0 <= va < NSS:
                    g_p, u_p = sched_attend[va]
                    if u_p == 0:
                        pa_tiles[2 * g_p] = ps_pa.tile([128, 512], f32, tag="pa", name=f"pa{2 * g_p}")
                        pa_tiles[2 * g_p + 1] = ps_pa.tile([128, 512], f32, tag="pa", name=f"pa{2 * g_p + 1}")
                    et_p = ets.pop((g_p, u_p))
                    for jp in range(2):
                        P = 2 * u_p + jp  # wsb pair: key chunks 2P, 2P+1
                        for h in range(2):
                            nc.tensor.matmul(
                                pa_tiles[2 * g_p + h][:, 0:257],
                                et_p[:, jp, :, 128 * h : 128 * h + 128],
                                wsb[:, P, :, 0:257],
                                start=(P == 0),
                                stop=(P == 15),
                                perf_mode=DR,
                            )
                    if u_p == 7:
                        for h in range(2):
                            epilogue(2 * g_p + h, pa_tiles[2 * g_p + h])
                            del pa_tiles[2 * g_p + h]

    nc.compile()
    return nc


def _get_compiled(use_bias: bool):
    key = bool(use_bias)
    if key not in _compiled_cache:
        _compiled_cache[key] = _build(use_bias)
    return _compiled_cache[key]


def _prep(x, wq, bq, wk, bk, wv, bv, wo, bo):
    xf = np.ascontiguousarray(np.asarray(x, dtype=np.float32)).reshape(B, N, C)
    wq = np.asarray(wq, np.float32)
    bq = np.asarray(bq, np.float32)
    wk = np.asarray(wk, np.float32)
    bk = np.asarray(bk, np.float32)
    wv = np.asarray(wv, np.float32)
    bv = np.asarray(bv, np.float32)
    wo = np.asarray(wo, np.float32)
    bo = np.asarray(bo, np.float32)

    use_bias = not (
        np.all(bq == 0) and np.all(bk == 0) and np.all(bv == 0) and np.all(bo == 0)
    )

    # Weights go in UNSCALED; the softmax scale and the band-contraction
    # factor are applied at the k evacuation.
    wq_rep = np.tile(wq, (1, 4)).astype(BF16)  # [256, 128]
    wk_rep = np.tile(wk, (1, 4)).astype(BF16)
    # fold wo into the value projection: W = x @ (wv@wo) + bv@wo
    wvo = (wv @ wo).astype(BF16)
    wblob = np.ascontiguousarray(
        np.concatenate(
            [wq_rep[0:128], wq_rep[128:256], wk_rep[0:128], wk_rep[128:256],
             wvo[0:128], wvo[128:256]],
            axis=1,
        )
    )  # [128, 1024]
    wbias = np.ascontiguousarray(
        np.concatenate(
            [np.tile(bq, 4), np.tile(bk, 4), bv @ wo], 0
        )[None, :]
    ).astype(BF16)  # [1, 512]

    in_maps = []
    for core in range(NCORES):
        b, h = divmod(core, 2)
        if h == 0:
            xo = xf[b]
        else:
            xo = np.concatenate([xf[b, NQ:], xf[b, :NQ]], 0)
        # channel-major transpose on host: [256, 4096] -> [128, 2, 4096]
        xT = np.ascontiguousarray(
            xo.T.reshape(2, 128, N).transpose(1, 0, 2).astype(BF16)
        )
        xq = np.ascontiguousarray(xo[:NQ])
        if use_bias:
            xq = xq + bo[None, :]
        im = {
            "xT": xT,
            "xq32": xq,
            "wblob": wblob,
        }
        if use_bias:
            im["wbias"] = wbias
        in_maps.append(im)
    return in_maps, use_bias


def _gather(results):
    out = np.empty((B, N, C), np.float32)
    for core in range(NCORES):
        b, h = divmod(core, 2)
        out[b, NQ * h : NQ * (h + 1)] = results[core]["out"]
    return out.reshape(B, HH, WW, C)


def kernel(x, wq, bq, wk, bk, wv, bv, wo, bo):
    from concourse.bass_utils import run_bass_kernel_spmd

    in_maps, use_bias = _prep(x, wq, bq, wk, bk, wv, bv, wo, bo)
    nc = _get_compiled(use_bias)
    res = run_bass_kernel_spmd(nc, in_maps, core_ids=list(range(NCORES)))
    return _gather(res.results)


def _ensure_ntff_hook():
    """The agent image's antenv stub lacks axon_hooks; synthesize it so
    run_bass_kernel_spmd(trace=True) can NTFF-profile via libaxon_pjrt."""
    import types

    try:
        from antenv.axon_hooks import get_axon_ntff_profile_hook  # noqa: F401
        return
    except ImportError:
        pass
    import antenv
    from trn_agent_boot.trn_boot import _ntff_profile_via_ctypes

    mod = types.ModuleType("antenv.axon_hooks")
    state = {"h": _ntff_profile_via_ctypes("/opt/axon/libaxon_pjrt.so")}
    mod.get_axon_ntff_profile_hook = lambda: state["h"]
    mod.set_axon_ntff_profile_hook = lambda h: state.__setitem__("h", h)
    sys.modules["antenv.axon_hooks"] = mod
    antenv.axon_hooks = mod


def run_traced(inputs, **kw):
    """For test.py: run with NTFF profiling; returns (output, BassKernelResults)."""
    from concourse.bass_utils import run_bass_kernel_spmd

    _ensure_ntff_hook()

    in_maps, use_bias = _prep(**inputs)
    nc = _get_compiled(use_bias)
    res = run_bass_kernel_spmd(nc, in_maps, core_ids=list(range(NCORES)), trace=True, **kw)
    return _gather(res.results), res


# revision 3
# speedup vs baseline: 1.0078x; 1.0078x over previous
"""Trainium2 Bass kernel for nn_AttentionBlock (B=4, H=W=64, C=256, D=32).

Sharding: 8 shards = 4 samples x 2 query-halves. Each core gets the full
sample (rows reordered so its 2048 query rows come first), computes K and
the fused V@Wo projection for all 4096 keys, and attention + residual for
its 2048 queries. No collectives needed.

v3 over the baseline:
  - Phase B (q/k and W=x@(wv@wo) projections) runs fp8-e4m3 DoubleRow: x
    arrives as one f8 [128,2,4096] tensor (half the DMA), each projection
    is a single DR matmul per chunk (half the PE time).
  - Scores run 2-way row-tiled (64-deep contraction = 2 replicated D=32
    bands): tiles (0,0)/(64,0) write different PSUM banks concurrently,
    roughly halving score matmul occupancy. Scores are emitted in
    2-superstep batches to bound tile-mode-switch drains.
  - The softmax exp stream is split across engines: most supersteps use
    the ScalarE Exp LUT (997ns per [128,1024] PSUM tile); 13 supersteps
    are offloaded to the idle VectorE via a Schraudolph bit-trick exp
    (bits16 = 128*log2e*(s-2) + 16256 -> bitcast bf16 -> cast f8), which
    is accuracy-neutral next to the existing f8 quantization of exp.
  - Epilogue folds (pa*rec)+x into one scalar_tensor_tensor op.
  - LAG 16 -> 8: the attend stream trails closer, cutting the drain tail.
"""

import sys

if "/opt/trn_rl_repo" not in sys.path:
    sys.path.insert(0, "/opt/trn_rl_repo")

import numpy as np
import ml_dtypes

BF16 = ml_dtypes.bfloat16
F8 = ml_dtypes.float8_e4m3

# Problem constants
B, HH, WW, C = 4, 64, 64, 256
D = 32
N = HH * WW          # 4096 keys per sample
NQ = N // 2          # 2048 queries per core
NCORES = 8
KC = N // 128        # 32 key chunks

LAG = 8              # attend trails the scores/exp stream by LAG supersteps
# Supersteps whose exp runs on VectorE (Schraudolph) instead of ScalarE.
# Spread over the steady region, away from epilogue supersteps (vv=8k+7+LAG).
DVE_EXP = frozenset({18, 21, 25, 28, 32, 35, 42, 45, 49, 52, 56, 59, 62})
SCH_A = float(128.0 * np.log2(np.e))            # 184.665
SCH_B = float(16256.0 - 2.0 * 128.0 * np.log2(np.e))  # bf16 bias, shift=2

_compiled_cache = {}


def _build(use_bias: bool):
    from contextlib import ExitStack
    from concourse import bacc, tile, mybir

    f32 = mybir.dt.float32
    bf = mybir.dt.bfloat16
    f8 = mybir.dt.float8e4
    i16 = mybir.dt.int16
    DR = mybir.MatmulPerfMode.DoubleRow

    nc = bacc.Bacc("TRN2", target_bir_lowering=False, debug=False, num_devices=NCORES)

    if use_bias:
        xT_d = nc.dram_tensor("xT", [128, 2, N], bf, kind="ExternalInput")
        wb_d = nc.dram_tensor("wblob", [128, 1024], bf, kind="ExternalInput")
        wbias_d = nc.dram_tensor("wbias", [1, 512], bf, kind="ExternalInput")
    else:
        xT8_d = nc.dram_tensor("xT8", [128, 2, N], f8, kind="ExternalInput")
        wb8_d = nc.dram_tensor("wblob8", [128, 2, 512], f8, kind="ExternalInput")
    xq32_d = nc.dram_tensor("xq32", [NQ, C], f32, kind="ExternalInput")
    out_d = nc.dram_tensor("out", [NQ, C], f32, kind="ExternalOutput")

    Exp = mybir.ActivationFunctionType.Exp
    # undo 2x band replication (64-deep tiled score contraction) + 1/sqrt(D)
    SC2 = float(1.0 / (2.0 * np.sqrt(np.float32(D))))
    Add = mybir.AluOpType.add
    Mult = mybir.AluOpType.mult

    with tile.TileContext(nc) as tc:
        with ExitStack() as ctx:
            const = ctx.enter_context(tc.tile_pool(name="const", bufs=1))
            big = ctx.enter_context(tc.tile_pool(name="big", bufs=1))
            xbp = ctx.enter_context(tc.tile_pool(name="xbp", bufs=3))
            expp = ctx.enter_context(tc.tile_pool(name="expp", bufs=16))
            small = ctx.enter_context(tc.tile_pool(name="small", bufs=2))
            # PSUM: 3 x [128,1024] working tiles (6 banks; shared by phase-B
            # projections and score matmuls) + 2 pa accumulator banks = 8.
            ps_sc = ctx.enter_context(tc.tile_pool(name="ps_sc", bufs=3, space="PSUM"))
            ps_pa = ctx.enter_context(tc.tile_pool(name="ps_pa", bufs=2, space="PSUM"))

            # ---- weights (one blob DMA) ----
            if use_bias:
                wall = const.tile([128, 1024], bf, tag="wall")
                nc.sync.dma_start(out=wall[:], in_=wb_d[:])
                wq0 = wall[:, 0:128]
                wq1 = wall[:, 128:256]
                wk0 = wall[:, 256:384]
                wk1 = wall[:, 384:512]
                wvo0 = wall[:, 512:768]
                wvo1 = wall[:, 768:1024]
                ones_row = const.tile([1, 512], bf, tag="ones_row")
                nc.gpsimd.memset(ones_row[:], 1.0)
                wbias = const.tile([1, 512], bf, tag="wbias")
                nc.sync.dma_start(out=wbias[:], in_=wbias_d[:])
                wqb = wbias[:, 0:128]
                wkb = wbias[:, 128:256]
                wvob = wbias[:, 256:512]
            else:
                w8all = const.tile([128, 2, 512], f8, tag="w8all")
                nc.sync.dma_start(out=w8all[:], in_=wb8_d[:])

            # Persistent SBUF: qT replicated across the 4 partition bands of
            # 32 (read as 2 bands of 64 by the tiled score matmuls), kT in 2
            # bands of 64 (band 0: chunks 4s/4s+1, band 1: chunks 4s+2/4s+3),
            # and W (= V@Wo) rows with a ones column at 256 for the softmax
            # denominator.
            qT4 = big.tile([128, NQ], bf, tag="qT4")
            kT4 = big.tile([128, N], bf, tag="kT4")
            wsb = big.tile([128, 16, 2, 272], f8, tag="wsb")
            nc.vector.memset(wsb[:, :, :, 256:257], 1.0)
            expbias = const.tile([128, 1], f32, tag="expbias")
            nc.vector.memset(expbias[:], -2.0)

            # ---- phase B inputs: DMA xT chunks ----
            xbs = []
            for s in range(8):
                if use_bias:
                    xb = xbp.tile([128, 2, 512], bf, tag="xb", bufs=8)
                    nc.sync.dma_start(out=xb[:], in_=xT_d[:, :, 512 * s : 512 * s + 512])
                else:
                    xb = xbp.tile([128, 2, 512], f8, tag="xb", bufs=8)
                    nc.sync.dma_start(out=xb[:], in_=xT8_d[:, :, 512 * s : 512 * s + 512])
                xbs.append(xb)

            def qk_chunk(s):
                # q (chunks 0-3) and k share one [128,1024] psum tile
                p = ps_sc.tile([128, 1024], f32, tag="sc", name=f"pqk{s}")
                if use_bias:
                    if s < 4:
                        nc.tensor.matmul(p[:, 0:512], wq0, xbs[s][:, 0, :], start=True, stop=False)
                        nc.tensor.matmul(p[:, 0:512], wq1, xbs[s][:, 1, :], start=False, stop=False)
                        nc.tensor.matmul(p[:, 0:512], wqb, ones_row[:], start=False, stop=True)
                    nc.tensor.matmul(p[:, 512:1024], wk0, xbs[s][:, 0, :], start=True, stop=False)
                    nc.tensor.matmul(p[:, 512:1024], wk1, xbs[s][:, 1, :], start=False, stop=False)
                    nc.tensor.matmul(p[:, 512:1024], wkb, ones_row[:], start=False, stop=True)
                else:
                    if s < 4:
                        nc.tensor.matmul(p[:, 0:512], w8all[:, :, 0:128], xbs[s][:], start=True, stop=True, perf_mode=DR)
                    nc.tensor.matmul(p[:, 512:1024], w8all[:, :, 128:256], xbs[s][:], start=True, stop=True, perf_mode=DR)
                if s < 4:
                    nc.vector.tensor_copy(qT4[:, 512 * s : 512 * s + 512], p[:, 0:512])
                # k carries the softmax scale and the 1/2 band-contraction
                # factor; band 0 <- chunks 4s,4s+1 ; band 1 <- 4s+2,4s+3
                # (psum k bands are 4 identical replicas, read diagonally)
                nc.vector.tensor_scalar(
                    kT4[0:64, 256 * s : 256 * s + 256], p[0:64, 512:768], SC2, None, Mult
                )
                nc.vector.tensor_scalar(
                    kT4[64:128, 256 * s : 256 * s + 256], p[64:128, 768:1024], SC2, None, Mult
                )

            def w_pass(s):
                # W = x @ (wv@wo): 4 key chunks of 128 per xb in one psum tile
                pw = ps_sc.tile([128, 1024], f32, tag="sc", name=f"pw{s}")
                for j2 in range(4):
                    off = 128 * j2
                    if use_bias:
                        nc.tensor.matmul(pw[:, 256 * j2 : 256 * j2 + 256], xbs[s][:, 0, off : off + 128], wvo0, start=True, stop=False)
                        nc.tensor.matmul(pw[:, 256 * j2 : 256 * j2 + 256], xbs[s][:, 1, off : off + 128], wvo1, start=False, stop=False)
                        nc.tensor.matmul(pw[:, 256 * j2 : 256 * j2 + 256], ones_row[:, 0:128], wvob, start=False, stop=True)
                    else:
                        nc.tensor.matmul(pw[:, 256 * j2 : 256 * j2 + 256], xbs[s][:, :, off : off + 128], w8all[:, :, 256:512], start=True, stop=True, perf_mode=DR)
                # evacuate on vector engine (scalar stays free for exps)
                nc.vector.tensor_copy(wsb[:, 2 * s : 2 * s + 2, :, 0:256], pw[:])

            def scores_mms(g, u):
                # 2-way row tiling: T0 (sbuf partitions 0-63) does chunks
                # 4u,4u+1 into bank A; T8 (64-127) does 4u+2,4u+3 into bank
                # B. Interleaved emission for cross-tile concurrency.
                pst = ps_sc.tile([128, 1024], f32, tag="sc", name=f"ps{g}_{u}")
                qs = slice(256 * g, 256 * g + 256)
                k0 = 256 * u
                nc.tensor.matmul(pst[:, 0:256], kT4[0:64, k0 : k0 + 128], qT4[0:64, qs], start=True, stop=True, tile_position=(0, 0))
                nc.tensor.matmul(pst[:, 512:768], kT4[64:128, k0 : k0 + 128], qT4[64:128, qs], start=True, stop=True, tile_position=(64, 0))
                nc.tensor.matmul(pst[:, 256:512], kT4[0:64, k0 + 128 : k0 + 256], qT4[0:64, qs], start=True, stop=True, tile_position=(0, 0))
                nc.tensor.matmul(pst[:, 768:1024], kT4[64:128, k0 + 128 : k0 + 256], qT4[64:128, qs], start=True, stop=True, tile_position=(64, 0))
                return pst

            def epilogue(qb, pa_t):
                rec = small.tile([128, 1], f32, tag="rec")
                nc.vector.reciprocal(rec[:], pa_t[:, 256:257])
                xq = small.tile([128, 256], f32, tag="xq", bufs=3)
                nc.sync.dma_start(out=xq[:], in_=xq32_d[128 * qb : 128 * qb + 128, :])
                ot = small.tile([128, 256], f32, tag="ot", bufs=3)
                # out = attended/denom + x in one DVE op
                nc.vector.scalar_tensor_tensor(ot[:], pa_t[:, 0:256], rec[:], xq[:], Mult, Add)
                nc.sync.dma_start(out=out_d[128 * qb : 128 * qb + 128, :], in_=ot[:])

            # ---- software pipeline ----
            # scores/exp stream order: chunk-diagonal over groups 0-1 during
            # the ramp (both only need q chunk 0 + k chunk u), then
            # group-major. Scores are emitted in PAIRS of supersteps (even
            # vv) so the PE switches tile mode at most twice per pair.
            pa_tiles = {}
            ets = {}
            pending_scores = {}
            NSS = 8 * 8
            sched_scores = [(g, u) for u in range(8) for g in (0, 1)] + [
                (g, u) for g in range(2, 8) for u in range(8)
            ]
            sched_attend = [(g, u) for g in range(8) for u in range(8)]

            def do_exp(vv):
                g, u = sched_scores[vv]
                pst = pending_scores.pop(vv)
                et = expp.tile([128, 2, 2, 256], f8, tag="e")
                if vv in DVE_EXP and not use_bias:
                    # Schraudolph exp on VectorE: bits16 = A*s + B (with the
                    # same -2 shift folded into B), bitcast bf16, cast f8.
                    # Exact-exp error vs this approx is far below the f8
                    # quantization both paths share.
                    etb = expp.tile([128, 1024], bf, tag="eb", bufs=3)
                    nc.vector.tensor_scalar(etb[:].bitcast(i16), pst[:], SCH_A, SCH_B, Mult, Add)
                    nc.vector.tensor_copy(et[:], etb[:])
                else:
                    # exp(s - 2): constant shift keeps exp within fp8-e4m3
                    # range (max score ~7 -> e^5 = 148 < 240); the
                    # ones-column denominator sees the same shift, so the
                    # normalized ratio is exact.
                    nc.scalar.activation(et[:], pst[:], Exp, bias=expbias[:])
                ets[(g, u)] = et

            for vv in range(NSS + LAG):
                if vv < 8:
                    qk_chunk(vv)
                if 4 <= vv < 12:
                    w_pass(vv - 4)
                if vv < NSS:
                    if vv % 2 == 0:
                        pending_scores[vv] = scores_mms(*sched_scores[vv])
                        if vv + 1 < NSS:
                            pending_scores[vv + 1] = scores_mms(*sched_scores[vv + 1])
                    do_exp(vv)
                va = vv - LAG
                if 0 <= va < NSS:
                    g_p, u_p = sched_attend[va]
                    if u_p == 0:
                        pa_tiles[2 * g_p] = ps_pa.tile([128, 512], f32, tag="pa", name=f"pa{2 * g_p}")
                        pa_tiles[2 * g_p + 1] = ps_pa.tile([128, 512], f32, tag="pa", name=f"pa{2 * g_p + 1}")
                    et_p = ets.pop((g_p, u_p))
                    for jp in range(2):
                        P = 2 * u_p + jp  # wsb pair: key chunks 2P, 2P+1
                        for h in range(2):
                            nc.tensor.matmul(
                                pa_tiles[2 * g_p + h][:, 0:257],
                                et_p[:, jp, :, 128 * h : 128 * h + 128],
                                wsb[:, P, :, 0:257],
                                start=(P == 0),
                                stop=(P == 15),
                                perf_mode=DR,
                            )
                    if u_p == 7:
                        for h in range(2):
                            epilogue(2 * g_p + h, pa_tiles[2 * g_p + h])
                            del pa_tiles[2 * g_p + h]

    nc.compile()
    return nc


def _get_compiled(use_bias: bool):
    key = bool(use_bias)
    if key not in _compiled_cache:
        _compiled_cache[key] = _build(use_bias)
    return _compiled_cache[key]


def _prep(x, wq, bq, wk, bk, wv, bv, wo, bo):
    xf = np.ascontiguousarray(np.asarray(x, dtype=np.float32)).reshape(B, N, C)
    wq = np.asarray(wq, np.float32)
    bq = np.asarray(bq, np.float32)
    wk = np.asarray(wk, np.float32)
    bk = np.asarray(bk, np.float32)
    wv = np.asarray(wv, np.float32)
    bv = np.asarray(bv, np.float32)
    wo = np.asarray(wo, np.float32)
    bo = np.asarray(bo, np.float32)

    use_bias = not (
        np.all(bq == 0) and np.all(bk == 0) and np.all(bv == 0) and np.all(bo == 0)
    )

    # Weights go in UNSCALED; the softmax scale and the band-contraction
    # factor are applied at the k evacuation.
    wvo = (wv @ wo).astype(BF16)  # fold wo into the value projection
    if use_bias:
        wq_rep = np.tile(wq, (1, 4)).astype(BF16)  # [256, 128]
        wk_rep = np.tile(wk, (1, 4)).astype(BF16)
        wblob = np.ascontiguousarray(
            np.concatenate(
                [wq_rep[0:128], wq_rep[128:256], wk_rep[0:128], wk_rep[128:256],
                 wvo[0:128], wvo[128:256]],
                axis=1,
            )
        )  # [128, 1024]
        wbias = np.ascontiguousarray(
            np.concatenate(
                [np.tile(bq, 4), np.tile(bk, 4), bv @ wo], 0
            )[None, :]
        ).astype(BF16)  # [1, 512]
    else:
        # fp8 DoubleRow packing: [ki, ko, col] = w[ki + 128*ko, col]
        wq_rep = np.tile(wq, (1, 4)).astype(np.float32)
        wk_rep = np.tile(wk, (1, 4)).astype(np.float32)
        blob = np.concatenate([wq_rep, wk_rep, wvo.astype(np.float32)], axis=1)  # [256, 512]
        wblob8 = np.ascontiguousarray(
            blob.reshape(2, 128, 512).transpose(1, 0, 2).astype(F8)
        )  # [128, 2, 512]

    in_maps = []
    for core in range(NCORES):
        b, h = divmod(core, 2)
        if h == 0:
            xo = xf[b]
        else:
            xo = np.concatenate([xf[b, NQ:], xf[b, :NQ]], 0)
        # channel-major transpose on host: [256, 4096] -> [128, 2, 4096]
        xT = np.ascontiguousarray(
            xo.T.reshape(2, 128, N).transpose(1, 0, 2)
        )
        xq = np.ascontiguousarray(xo[:NQ])
        if use_bias:
            xq = xq + bo[None, :]
            im = {"xT": xT.astype(BF16), "xq32": xq, "wblob": wblob, "wbias": wbias}
        else:
            im = {"xT8": xT.astype(F8), "xq32": xq, "wblob8": wblob8}
        in_maps.append(im)
    return in_maps, use_bias


def _gather(results):
    out = np.empty((B, N, C), np.float32)
    for core in range(NCORES):
        b, h = divmod(core, 2)
        out[b, NQ * h : NQ * (h + 1)] = results[core]["out"]
    return out.reshape(B, HH, WW, C)


def kernel(x, wq, bq, wk, bk, wv, bv, wo, bo):
    from concourse.bass_utils import run_bass_kernel_spmd

    in_maps, use_bias = _prep(x, wq, bq, wk, bk, wv, bv, wo, bo)
    nc = _get_compiled(use_bias)
    res = run_bass_kernel_spmd(nc, in_maps, core_ids=list(range(NCORES)))
    return _gather(res.results)


def _ensure_ntff_hook():
    """The agent image's antenv stub lacks axon_hooks; synthesize it so
    run_bass_kernel_spmd(trace=True) can NTFF-profile via libaxon_pjrt."""
    import types

    try:
        from antenv.axon_hooks import get_axon_ntff_profile_hook  # noqa: F401
        return
    except ImportError:
        pass
    import antenv
    from trn_agent_boot.trn_boot import _ntff_profile_via_ctypes

    mod = types.ModuleType("antenv.axon_hooks")
    state = {"h": _ntff_profile_via_ctypes("/opt/axon/libaxon_pjrt.so")}
    mod.get_axon_ntff_profile_hook = lambda: state["h"]
    mod.set_axon_ntff_profile_hook = lambda h: state.__setitem__("h", h)
    sys.modules["antenv.axon_hooks"] = mod
    antenv.axon_hooks = mod


def run_traced(inputs, **kw):
    """For test.py: run with NTFF profiling; returns (output, BassKernelResults)."""
    from concourse.bass_utils import run_bass_kernel_spmd

    _ensure_ntff_hook()

    in_maps, use_bias = _prep(**inputs)
    nc = _get_compiled(use_bias)
    res = run_bass_kernel_spmd(nc, in_maps, core_ids=list(range(NCORES)), trace=True, **kw)
    return _gather(res.results), res
